# revision 6
# baseline (speedup 1.0000x reference)
"""Trainium2 Bass kernel for nn_BucketedGoWatti (sparse windowed attention pooling).

Chunk-sum reformulation: e_r = exp(qk . H_r * s) is window-independent, so
per-window numerators/denominators are sliding sums of 256-row chunk
partials (host). Device per core (batch, 2048-row half):
  logits = qkT.T @ HT (fp8 H^T), explt = exp(transpose(logits)),
  CS = explt_pad.T @ Hn with 4 chunks batched per matmul group
  (zero-padded stacked stationary -> out partitions = 4 chunks x T),
  csums = explt_pad.T @ ones.
DMA pipeline on the SP ring: HT half-quarters (qkT after the first),
then Hn chunk pieces; CS group matmuls chase the Hn pieces; a 6-matmul
heater warms the PE clock during the DMA lead-in; outputs overlap.
"""
import numpy as np
import ml_dtypes
from contextlib import ExitStack

import concourse.bacc as bacc
import concourse.tile as tile
import concourse.mybir as mybir
import concourse.masks as masks
from concourse.bass_utils import run_bass_kernel_spmd

F32 = mybir.dt.float32
F32R = mybir.dt.float32r
BF16 = mybir.dt.bfloat16
F8 = mybir.dt.float8e3
ActFn = mybir.ActivationFunctionType

B, L, T = 4, 4096, 32
DH, DG, DP = 1024, 256, 256
WIN, STRIDE = 1024, 256
W = (L - WIN) // STRIDE + 1          # 13
R = L // 2                           # 2048 rows per core
NLT = R // 128                       # 16 l-tiles
NCH = R // 256                       # 8 chunks per core
NQ = 4                               # logits quarters (512 rows each)
NG = 2                               # CS groups of 4 chunks
NDT = DH // 128                      # 8 d-tiles
S_CORE = 1.0 / float(np.sqrt(DP))
S_WIN = 1.0 / float(np.sqrt(DH))

_CACHE = {}


def _build(with_mask: bool):
    nc = bacc.Bacc("TRN2", debug=False, target_bir_lowering=False)

    Hn_d = nc.dram_tensor("Hn", [R, DH], F8, kind="ExternalInput")
    HT_d = nc.dram_tensor("HT", [DH, R], F8, kind="ExternalInput")
    qkT_d = nc.dram_tensor("qkT", [DH, T], BF16, kind="ExternalInput")
    if with_mask:
        mb_d = nc.dram_tensor("maskbias", [1, R], F32R, kind="ExternalInput")
        ones_d = nc.dram_tensor("onesrow", [1, T], F32R, kind="ExternalInput")
    cs_d = nc.dram_tensor("CS_out", [NCH * T, DH], BF16, kind="ExternalOutput")
    lg_d = nc.dram_tensor("lg_out", [T, R], BF16, kind="ExternalOutput")

    with tile.TileContext(nc) as tc, ExitStack() as ctx:
        const = ctx.enter_context(tc.tile_pool(name="const", bufs=1))
        hq = ctx.enter_context(tc.tile_pool(name="hq", bufs=1))
        sb = ctx.enter_context(tc.tile_pool(name="sb", bufs=1))
        lg = ctx.enter_context(tc.tile_pool(name="lg", bufs=2, space="PSUM"))
        trp = ctx.enter_context(tc.tile_pool(name="trp", bufs=2, space="PSUM"))
        csp = ctx.enter_context(tc.tile_pool(name="csp", bufs=4, space="PSUM"))

        # ---- small inputs / constants; qkT first on the ACT HWDGE ring ----
        qkt = const.tile([128, NDT * T], BF16, tag="qkt")
        identb = const.tile([T, T], BF16, tag="identb")
        masks.make_identity(nc, identb[:])
        onesb = const.tile([128, 1], BF16, tag="onesb")
        nc.gpsimd.memset(onesb[:], 1.0)
        heat = const.tile([128, 512], BF16, tag="heat")
        nc.gpsimd.memset(heat[:], 0.0)
        for _ in range(8):
            ps_h = lg.tile([1, 512], F32, tag="lg")
            nc.tensor.matmul(ps_h[:], onesb[:], heat[:], start=True, stop=True)
        if with_mask:
            mbias = const.tile([1, R], F32R, tag="mbias")
            onesr = const.tile([1, T], F32R, tag="onesr")
            nc.gpsimd.dma_start(mbias[:], mb_d.ap())
            nc.gpsimd.dma_start(onesr[:], ones_d.ap())

        # zero-padded stacked exp tiles, one per group of 4 chunks
        explt_g = []
        for g in range(NG):
            t_ = sb.tile([128, 8 * 128], BF16, tag=f"explt{g}")
            nc.gpsimd.memset(t_[:], 0.0)
            explt_g.append(t_)

        # ---- input loads on SP ring: HT quarters, then Hn chunk pieces ----
        ht_q = []
        for q in range(NQ):
            halves = []
            for v in range(2):
                ht_ = hq.tile([128, 4 * 512], F8, tag=f"ht{q}_{v}")
                nc.sync.dma_start(
                    ht_[:], HT_d.ap()[512 * v:512 * (v + 1),
                                      512 * q:512 * (q + 1)]
                    .rearrange("(i p) l -> p i l", p=128))
                halves.append(ht_)
                if q == 0 and v == 0:
                    nc.sync.dma_start(
                        qkt[:], qkT_d.ap().rearrange("(i p) t -> p i t", p=128))
            ht_q.append(halves)
        hn_t = []                     # one tile handle per l-tile [128, DH]
        for cc in range(NCH):
            if cc < NCH - 1:
                hn_ = hq.tile([128, 2 * DH], F8, tag=f"hn{cc}")
                nc.sync.dma_start(
                    hn_[:], Hn_d.ap()[256 * cc:256 * (cc + 1), :]
                    .rearrange("(j p) d -> p j d", p=128))
                hn_t.append(hn_[:, :DH])
                hn_t.append(hn_[:, DH:])
            else:                     # split the last piece for a shorter tail
                for k in range(2):
                    hn_ = hq.tile([128, DH], F8, tag=f"hn{cc}_{k}")
                    r0 = 256 * cc + 128 * k
                    nc.sync.dma_start(
                        hn_[:], Hn_d.ap()[r0:r0 + 128, :])
                    hn_t.append(hn_[:])

        # ---- interleaved schedule: logits quarters, transposes chasing two
        # quarters behind, CS groups chasing the Hn pieces ----
        lgbs = [None] * NQ
        explt_done = [False] * NLT

        def emit_logits(q):
            ps_lg = lg.tile([T, 512], F32, tag="lg")
            for i in range(NDT):
                nc.tensor.matmul(ps_lg[:], qkt[:, i * T:(i + 1) * T],
                                 ht_q[q][i // 4][:, (i % 4) * 512:(i % 4 + 1) * 512],
                                 start=(i == 0),
                                 stop=(i == NDT - 1 and not with_mask))
            if with_mask:
                nc.tensor.matmul(ps_lg[:], onesr[:],
                                 mbias[:, q * 512:(q + 1) * 512],
                                 start=False, stop=True)
            lgb = sb.tile([T, 512], BF16, tag=f"lgb{q}")
            nc.vector.tensor_copy(lgb[:], ps_lg[:])
            nc.gpsimd.dma_start(lg_d.ap()[:, q * 512:(q + 1) * 512], lgb[:])
            lgbs[q] = lgb

        def emit_logits_pair(w):
            # two quarters col-tiled into PE column-groups 0/32: both M=32
            # matmuls of a d-step run concurrently in the 128x128 array
            ps_lg = lg.tile([2 * T, 512], F32, tag="lg")
            for i in range(NDT):
                for u in range(2):
                    q = 2 * w + u
                    nc.tensor.matmul(
                        ps_lg[u * T:(u + 1) * T, :], qkt[:, i * T:(i + 1) * T],
                        ht_q[q][i // 4][:, (i % 4) * 512:(i % 4 + 1) * 512],
                        start=(i == 0), stop=(i == NDT - 1),
                        tile_position=(0, u * T))
            for u in range(2):
                q = 2 * w + u
                lgb = sb.tile([T, 512], BF16, tag=f"lgb{q}")
                nc.vector.tensor_copy(lgb[:], ps_lg[u * T:(u + 1) * T, :])
                nc.gpsimd.dma_start(lg_d.ap()[:, q * 512:(q + 1) * 512], lgb[:])
                lgbs[q] = lgb

        def emit_tr(q):
            ps_tr = trp.tile([128, 4 * T], BF16, tag="tr")
            for jj in range(4):
                nc.tensor.transpose(ps_tr[:, jj * T:(jj + 1) * T],
                                    lgbs[q][:, jj * 128:(jj + 1) * 128],
                                    identb[:])
            for jj in range(4):
                j = 4 * q + jj                       # l-tile index
                g, b = j // 8, (j // 2) % 4          # group, chunk block
                nc.scalar.activation(
                    explt_g[g][:, (j % 8) * 128 + b * T:(j % 8) * 128 + (b + 1) * T],
                    ps_tr[:, jj * T:(jj + 1) * T], ActFn.Exp)

        cs_ps = {}

        def emit_cs(g, j8s):
            if g not in cs_ps:
                psA = csp.tile([128, 512], F32, tag="cs")
                psB = csp.tile([128, 512], F32, tag="cs")
                cs_ps[g] = (psA, psB)
            psA, psB = cs_ps[g]
            for j8 in j8s:
                j = 8 * g + j8
                lhs = explt_g[g][:, j8 * 128:(j8 + 1) * 128]
                for h in range(2):
                    nc.tensor.matmul(
                        [psA, psB][h][:], lhs,
                        hn_t[j][:, h * 512:(h + 1) * 512],
                        start=(j8 == 0), stop=(j8 == 7))

        def emit_out(g, hwdge=False):
            psA, psB = cs_ps[g]
            csoutA = sb.tile([128, 512], BF16, tag=f"csoutA{g}")
            csoutB = sb.tile([128, 512], BF16, tag=f"csoutB{g}")
            nc.vector.tensor_copy(csoutA[:], psA[:])
            nc.scalar.activation(csoutB[:], psB[:], ActFn.Identity)
            engB = nc.sync if hwdge else nc.scalar
            nc.gpsimd.dma_start(cs_d.ap()[g * 128:(g + 1) * 128, :512], csoutA[:])
            engB.dma_start(cs_d.ap()[g * 128:(g + 1) * 128, 512:], csoutB[:])

        if with_mask:
            emit_logits(0)
            emit_logits(1)
            emit_tr(0)
            emit_logits(2)
            emit_tr(1)
            emit_logits(3)
            emit_tr(2)
        else:
            emit_logits_pair(0)
            emit_tr(0)
            emit_logits_pair(1)
            emit_tr(1)
            emit_tr(2)
        emit_cs(0, range(0, 6))
        emit_tr(3)
        emit_cs(0, range(6, 8))
        emit_out(0)
        emit_cs(1, range(0, 4))
        emit_cs(1, range(4, 8))
        emit_out(1, hwdge=True)


    nc.compile()
    return nc


def _host_prep(H, G, Wq_core, Wk_core, Wq_win, Wk_win):
    qk = np.einsum("btg,gp->btp", G, Wq_core) @ Wk_core.T * S_CORE   # [B,T,DH]
    qkT = np.ascontiguousarray(qk.transpose(0, 2, 1)).astype(ml_dtypes.bfloat16)
    qw2 = np.einsum("btg,gd->btd", G, Wq_win) @ Wk_win.T * S_WIN     # [B,T,DH]
    Hb = H.astype(ml_dtypes.float8_e3m4)
    HT8 = np.ascontiguousarray(H.transpose(0, 2, 1)).astype(ml_dtypes.float8_e3m4)
    return qkT, qw2, Hb, HT8


def kernel(H, G, Wq_core, Wk_core, Wq_win, Wk_win, attn_mask):
    H = np.asarray(H, dtype=np.float32)
    G = np.asarray(G, dtype=np.float32)
    Wq_core = np.asarray(Wq_core, dtype=np.float32)
    Wk_core = np.asarray(Wk_core, dtype=np.float32)
    Wq_win = np.asarray(Wq_win, dtype=np.float32)
    Wk_win = np.asarray(Wk_win, dtype=np.float32)
    mask = np.asarray(attn_mask).astype(bool)

    with_mask = not bool(mask.all())
    key = ("k", with_mask)
    if key not in _CACHE:
        _CACHE[key] = _build(with_mask)
    nc = _CACHE[key]

    qkT, qw2, Hb, HT8 = _host_prep(H, G, Wq_core, Wk_core, Wq_win, Wk_win)

    in_maps = []
    for c in range(8):
        b, half = c // 2, c % 2
        lo = half * R
        im = {
            "Hn": np.ascontiguousarray(Hb[b, lo:lo + R, :]),
            "HT": np.ascontiguousarray(HT8[b, :, lo:lo + R]),
            "qkT": qkT[b],
        }
        if with_mask:
            im["maskbias"] = np.where(mask[b, lo:lo + R], 0.0,
                                      -1e9).astype(np.float32)[None, :]
            im["onesrow"] = np.ones((1, T), dtype=np.float32)
        in_maps.append(im)

    import os
    prof_dir = os.environ.get("BGW_PROFILE_DIR")
    if prof_dir:
        try:
            res = run_bass_kernel_spmd(nc, in_maps, core_ids=list(range(8)),
                                       trace=True, tmpdir=prof_dir)
        except (ImportError, ModuleNotFoundError):
            res = run_bass_kernel_spmd(nc, in_maps, core_ids=list(range(8)))
    else:
        res = run_bass_kernel_spmd(nc, in_maps, core_ids=list(range(8)))
    kernel._last_result = res

    # ---- host combine: sliding window sums + tiny cross-window softmax ----
    NC2 = 2 * NCH                                     # 16 chunks per batch
    Z = np.empty((B, T, DH), dtype=np.float32)
    for b in range(B):
        CS = np.empty((NC2, T, DH), dtype=np.float32)
        csum = np.empty((T, NC2), dtype=np.float32)
        for half in range(2):
            r = res.results[2 * b + half]
            CS[half * NCH:(half + 1) * NCH] = (
                r["CS_out"].astype(np.float32).reshape(NCH, T, DH))
            e = np.exp(r["lg_out"].astype(np.float32))
            csum[:, half * NCH:(half + 1) * NCH] = (
                e.reshape(T, NCH, 256).sum(-1))
        csl = np.cumsum(CS, axis=0)                   # [16,T,DH]
        ZwN = np.stack([csl[w + 3] - (csl[w - 1] if w else 0) for w in range(W)])
        cl = np.cumsum(csum, axis=1)
        den = np.stack([cl[:, w + 3] - (cl[:, w - 1] if w else 0)
                        for w in range(W)], axis=0)   # [W,T]
        Zw = ZwN / den[:, :, None]
        wlog = np.einsum("wtd,td->tw", Zw, qw2[b])
        m = wlog.max(axis=1, keepdims=True)
        e = np.exp(wlog - m)
        wsm = e / e.sum(axis=1, keepdims=True)        # [T,W]
        Z[b] = np.einsum("tw,wtd->td", wsm, Zw)
    return Z


# revision 7
# speedup vs baseline: 1.0024x; 1.0024x over previous
"""Trainium2 Bass kernel for nn_BucketedGoWatti (sparse windowed attention pooling).

Chunk-sum reformulation: e_r = exp(qk . H_r * s) is window-independent, so
per-window numerators/denominators are sliding sums of 256-row chunk
partials (host). Device per core (batch, 2048-row half):
  logits = qkT.T @ HT (fp8 H^T), explt = exp(transpose(logits)),
  CS = explt_pad.T @ Hn with 4 chunks batched per matmul group
  (zero-padded stacked stationary -> out partitions = 4 chunks x T),
  csums = explt_pad.T @ ones.
DMA pipeline on the SP ring: HT half-quarters (qkT after the first),
then Hn chunk pieces; CS group matmuls chase the Hn pieces; a 6-matmul
heater warms the PE clock during the DMA lead-in; outputs overlap.
"""
import numpy as np
import ml_dtypes
from contextlib import ExitStack

import concourse.bacc as bacc
import concourse.tile as tile
import concourse.mybir as mybir
import concourse.masks as masks
from concourse.bass_utils import run_bass_kernel_spmd

F32 = mybir.dt.float32
F32R = mybir.dt.float32r
BF16 = mybir.dt.bfloat16
F8 = mybir.dt.float8e3
ActFn = mybir.ActivationFunctionType

B, L, T = 4, 4096, 32
DH, DG, DP = 1024, 256, 256
WIN, STRIDE = 1024, 256
W = (L - WIN) // STRIDE + 1          # 13
R = L // 2                           # 2048 rows per core
NLT = R // 128                       # 16 l-tiles
NCH = R // 256                       # 8 chunks per core
NQ = 4                               # logits quarters (512 rows each)
NG = 2                               # CS groups of 4 chunks
NDT = DH // 128                      # 8 d-tiles
S_CORE = 1.0 / float(np.sqrt(DP))
S_WIN = 1.0 / float(np.sqrt(DH))

_CACHE = {}


def _build(with_mask: bool):
    nc = bacc.Bacc("TRN2", debug=False, target_bir_lowering=False)

    Hn_d = nc.dram_tensor("Hn", [R, DH], F8, kind="ExternalInput")
    HT_d = nc.dram_tensor("HT", [DH, R], F8, kind="ExternalInput")
    qkT_d = nc.dram_tensor("qkT", [DH, T], BF16, kind="ExternalInput")
    if with_mask:
        mb_d = nc.dram_tensor("maskbias", [1, R], F32R, kind="ExternalInput")
        ones_d = nc.dram_tensor("onesrow", [1, T], F32R, kind="ExternalInput")
    cs_d = nc.dram_tensor("CS_out", [NCH * T, DH], BF16, kind="ExternalOutput")
    lg_d = nc.dram_tensor("lg_out", [T, R], BF16, kind="ExternalOutput")

    with tile.TileContext(nc) as tc, ExitStack() as ctx:
        const = ctx.enter_context(tc.tile_pool(name="const", bufs=1))
        hq = ctx.enter_context(tc.tile_pool(name="hq", bufs=1))
        sb = ctx.enter_context(tc.tile_pool(name="sb", bufs=1))
        lg = ctx.enter_context(tc.tile_pool(name="lg", bufs=2, space="PSUM"))
        trp = ctx.enter_context(tc.tile_pool(name="trp", bufs=2, space="PSUM"))
        csp = ctx.enter_context(tc.tile_pool(name="csp", bufs=4, space="PSUM"))

        # ---- small inputs / constants; qkT first on the ACT HWDGE ring ----
        qkt = const.tile([128, NDT * T], BF16, tag="qkt")
        identb = const.tile([T, T], BF16, tag="identb")
        masks.make_identity(nc, identb[:])
        onesb = const.tile([128, 1], BF16, tag="onesb")
        nc.gpsimd.memset(onesb[:], 1.0)
        heat = const.tile([128, 512], BF16, tag="heat")
        nc.gpsimd.memset(heat[:], 0.0)
        for _ in range(8):
            ps_h = lg.tile([1, 512], F32, tag="lg")
            nc.tensor.matmul(ps_h[:], onesb[:], heat[:], start=True, stop=True)
        if with_mask:
            mbias = const.tile([1, R], F32R, tag="mbias")
            onesr = const.tile([1, T], F32R, tag="onesr")
            nc.gpsimd.dma_start(mbias[:], mb_d.ap())
            nc.gpsimd.dma_start(onesr[:], ones_d.ap())

        # zero-padded stacked exp tiles, one per group of 4 chunks
        explt_g = []
        for g in range(NG):
            t_ = sb.tile([128, 8 * 128], BF16, tag=f"explt{g}")
            nc.gpsimd.memset(t_[:], 0.0)
            explt_g.append(t_)

        # ---- input loads on SP ring: HT quarters, then Hn chunk pieces ----
        ht_q = []
        for q in range(NQ):
            halves = []
            for v in range(2):
                ht_ = hq.tile([128, 4 * 512], F8, tag=f"ht{q}_{v}")
                nc.sync.dma_start(
                    ht_[:], HT_d.ap()[512 * v:512 * (v + 1),
                                      512 * q:512 * (q + 1)]
                    .rearrange("(i p) l -> p i l", p=128))
                halves.append(ht_)
                if q == 0 and v == 0:
                    nc.sync.dma_start(
                        qkt[:], qkT_d.ap().rearrange("(i p) t -> p i t", p=128))
            ht_q.append(halves)
        hn_t = []                     # one tile handle per l-tile [128, DH]
        for cc in range(NCH):
            if cc < NCH - 1:
                hn_ = hq.tile([128, 2 * DH], F8, tag=f"hn{cc}")
                nc.sync.dma_start(
                    hn_[:], Hn_d.ap()[256 * cc:256 * (cc + 1), :]
                    .rearrange("(j p) d -> p j d", p=128))
                hn_t.append(hn_[:, :DH])
                hn_t.append(hn_[:, DH:])
            else:                     # split the last piece for a shorter tail
                for k in range(2):
                    hn_ = hq.tile([128, DH], F8, tag=f"hn{cc}_{k}")
                    r0 = 256 * cc + 128 * k
                    nc.sync.dma_start(
                        hn_[:], Hn_d.ap()[r0:r0 + 128, :])
                    hn_t.append(hn_[:])

        # ---- interleaved schedule: logits quarters, transposes chasing two
        # quarters behind, CS groups chasing the Hn pieces ----
        lgbs = [None] * NQ
        explt_done = [False] * NLT

        def emit_logits(q):
            ps_lg = lg.tile([T, 512], F32, tag="lg")
            for i in range(NDT):
                nc.tensor.matmul(ps_lg[:], qkt[:, i * T:(i + 1) * T],
                                 ht_q[q][i // 4][:, (i % 4) * 512:(i % 4 + 1) * 512],
                                 start=(i == 0),
                                 stop=(i == NDT - 1 and not with_mask))
            if with_mask:
                nc.tensor.matmul(ps_lg[:], onesr[:],
                                 mbias[:, q * 512:(q + 1) * 512],
                                 start=False, stop=True)
            lgb = sb.tile([T, 512], BF16, tag=f"lgb{q}")
            nc.vector.tensor_copy(lgb[:], ps_lg[:])
            nc.gpsimd.dma_start(lg_d.ap()[:, q * 512:(q + 1) * 512], lgb[:])
            lgbs[q] = lgb

        def emit_logits_pair(w):
            # two quarters col-tiled into PE column-groups 0/32: both M=32
            # matmuls of a d-step run concurrently in the 128x128 array
            ps_lg = lg.tile([2 * T, 512], F32, tag="lg")
            for i in range(NDT):
                for u in range(2):
                    q = 2 * w + u
                    nc.tensor.matmul(
                        ps_lg[u * T:(u + 1) * T, :], qkt[:, i * T:(i + 1) * T],
                        ht_q[q][i // 4][:, (i % 4) * 512:(i % 4 + 1) * 512],
                        start=(i == 0), stop=(i == NDT - 1),
                        tile_position=(0, u * T))
            for u in range(2):
                q = 2 * w + u
                lgb = sb.tile([T, 512], BF16, tag=f"lgb{q}")
                nc.vector.tensor_copy(lgb[:], ps_lg[u * T:(u + 1) * T, :])
                nc.gpsimd.dma_start(lg_d.ap()[:, q * 512:(q + 1) * 512], lgb[:])
                lgbs[q] = lgb

        def emit_tr(q):
            ps_tr = trp.tile([128, 4 * T], BF16, tag="tr")
            for jj in range(4):
                nc.tensor.transpose(ps_tr[:, jj * T:(jj + 1) * T],
                                    lgbs[q][:, jj * 128:(jj + 1) * 128],
                                    identb[:])
            for jj in range(4):
                j = 4 * q + jj                       # l-tile index
                g, b = j // 8, (j // 2) % 4          # group, chunk block
                nc.scalar.activation(
                    explt_g[g][:, (j % 8) * 128 + b * T:(j % 8) * 128 + (b + 1) * T],
                    ps_tr[:, jj * T:(jj + 1) * T], ActFn.Exp)

        cs_ps = {}

        def emit_cs(g, j8s):
            if g not in cs_ps:
                psA = csp.tile([128, 512], F32, tag="cs")
                psB = csp.tile([128, 512], F32, tag="cs")
                cs_ps[g] = (psA, psB)
            psA, psB = cs_ps[g]
            for j8 in j8s:
                j = 8 * g + j8
                lhs = explt_g[g][:, j8 * 128:(j8 + 1) * 128]
                for h in range(2):
                    nc.tensor.matmul(
                        [psA, psB][h][:], lhs,
                        hn_t[j][:, h * 512:(h + 1) * 512],
                        start=(j8 == 0), stop=(j8 == 7))

        def emit_out(g, hwdge=False):
            psA, psB = cs_ps[g]
            csoutA = sb.tile([128, 512], BF16, tag=f"csoutA{g}")
            csoutB = sb.tile([128, 512], BF16, tag=f"csoutB{g}")
            nc.vector.tensor_copy(csoutA[:], psA[:])
            nc.scalar.activation(csoutB[:], psB[:], ActFn.Identity)
            engB = nc.sync if hwdge else nc.scalar
            nc.gpsimd.dma_start(cs_d.ap()[g * 128:(g + 1) * 128, :512], csoutA[:])
            engB.dma_start(cs_d.ap()[g * 128:(g + 1) * 128, 512:], csoutB[:])

        if with_mask:
            emit_logits(0)
            emit_logits(1)
            emit_tr(0)
            emit_logits(2)
            emit_tr(1)
            emit_logits(3)
            emit_tr(2)
        else:
            emit_logits_pair(0)
            emit_tr(0)
            emit_tr(1)
            emit_logits_pair(1)
            emit_tr(2)
            emit_tr(3)
        emit_cs(0, range(0, 8))
        emit_out(0)
        emit_cs(1, range(0, 8))
        emit_out(1, hwdge=True)


    nc.compile()
    return nc


def _host_prep(H, G, Wq_core, Wk_core, Wq_win, Wk_win):
    qk = np.einsum("btg,gp->btp", G, Wq_core) @ Wk_core.T * S_CORE   # [B,T,DH]
    qkT = np.ascontiguousarray(qk.transpose(0, 2, 1)).astype(ml_dtypes.bfloat16)
    qw2 = np.einsum("btg,gd->btd", G, Wq_win) @ Wk_win.T * S_WIN     # [B,T,DH]
    Hb = H.astype(ml_dtypes.float8_e3m4)
    HT8 = np.ascontiguousarray(H.transpose(0, 2, 1)).astype(ml_dtypes.float8_e3m4)
    return qkT, qw2, Hb, HT8


def kernel(H, G, Wq_core, Wk_core, Wq_win, Wk_win, attn_mask):
    H = np.asarray(H, dtype=np.float32)
    G = np.asarray(G, dtype=np.float32)
    Wq_core = np.asarray(Wq_core, dtype=np.float32)
    Wk_core = np.asarray(Wk_core, dtype=np.float32)
    Wq_win = np.asarray(Wq_win, dtype=np.float32)
    Wk_win = np.asarray(Wk_win, dtype=np.float32)
    mask = np.asarray(attn_mask).astype(bool)

    with_mask = not bool(mask.all())
    key = ("k", with_mask)
    if key not in _CACHE:
        _CACHE[key] = _build(with_mask)
    nc = _CACHE[key]

    qkT, qw2, Hb, HT8 = _host_prep(H, G, Wq_core, Wk_core, Wq_win, Wk_win)

    in_maps = []
    for c in range(8):
        b, half = c // 2, c % 2
        lo = half * R
        im = {
            "Hn": np.ascontiguousarray(Hb[b, lo:lo + R, :]),
            "HT": np.ascontiguousarray(HT8[b, :, lo:lo + R]),
            "qkT": qkT[b],
        }
        if with_mask:
            im["maskbias"] = np.where(mask[b, lo:lo + R], 0.0,
                                      -1e9).astype(np.float32)[None, :]
            im["onesrow"] = np.ones((1, T), dtype=np.float32)
        in_maps.append(im)

    import os
    prof_dir = os.environ.get("BGW_PROFILE_DIR")
    if prof_dir:
        try:
            res = run_bass_kernel_spmd(nc, in_maps, core_ids=list(range(8)),
                                       trace=True, tmpdir=prof_dir)
        except (ImportError, ModuleNotFoundError):
            res = run_bass_kernel_spmd(nc, in_maps, core_ids=list(range(8)))
    else:
        res = run_bass_kernel_spmd(nc, in_maps, core_ids=list(range(8)))
    kernel._last_result = res

    # ---- host combine: sliding window sums + tiny cross-window softmax ----
    NC2 = 2 * NCH                                     # 16 chunks per batch
    Z = np.empty((B, T, DH), dtype=np.float32)
    for b in range(B):
        CS = np.empty((NC2, T, DH), dtype=np.float32)
        csum = np.empty((T, NC2), dtype=np.float32)
        for half in range(2):
            r = res.results[2 * b + half]
            CS[half * NCH:(half + 1) * NCH] = (
                r["CS_out"].astype(np.float32).reshape(NCH, T, DH))
            e = np.exp(r["lg_out"].astype(np.float32))
            csum[:, half * NCH:(half + 1) * NCH] = (
                e.reshape(T, NCH, 256).sum(-1))
        csl = np.cumsum(CS, axis=0)                   # [16,T,DH]
        ZwN = np.stack([csl[w + 3] - (csl[w - 1] if w else 0) for w in range(W)])
        cl = np.cumsum(csum, axis=1)
        den = np.stack([cl[:, w + 3] - (cl[:, w - 1] if w else 0)
                        for w in range(W)], axis=0)   # [W,T]
        Zw = ZwN / den[:, :, None]
        wlog = np.einsum("wtd,td->tw", Zw, qw2[b])
        m = wlog.max(axis=1, keepdims=True)
        e = np.exp(wlog - m)
        wsm = e / e.sum(axis=1, keepdims=True)        # [T,W]
        Z[b] = np.einsum("tw,wtd->td", wsm, Zw)
    return Z
